# revision 64
# baseline (speedup 1.0000x reference)
"""Bass/Trainium2 SPMD kernel for a causal attention layer.

Problem: hidden [2, 2048, 1024], W_attn [1024, 3072], W_proj [1024, 1024],
H=16 heads, head_dim=64, causal softmax attention + output projection.

Sharding (8 cores): core c handles batch c//4 and head-group c%4 (4 heads).
Each core computes attention for its 4 heads plus the matching partial
output projection (W_proj row-sharded); the host sums the 4 partials per
batch - the unshard step of a row-sharded tensor-parallel projection.

Device algorithm (per core), all activations transposed (seq on the free
dim) so no on-chip transposes are ever needed; PE matmuls in bf16,
accumulation in fp32 PSUM:
  hT [D, S] bf16      host-pretransposed hidden^T, streamed in 4 DMAs
                      chunked along the SEQUENCE so chunk-0 projections
                      start after ~1MB instead of after the full 4MB
  Q^T/K^T [128, S]    per head-pair: 2 heads x 64 dims on the partitions
  V'' [128, 256] bf16 per key-tile: [V_even | ones64 | V_odd | ones64];
                      the ones-columns make the PV matmul emit the softmax
                      denominator replicated on PSUM rows 64..127
  scores^T [128 keys, 1024] in a 2-bank PSUM tile (head-even | head-odd),
  one ACT exp per key-tile; causal mask = one bf16 multiply with a
  host-built mask tile; 1/l = exp(-ln(l)) on ACT.

Schedule shaping (tuned against the NTFF profile): the k-loop is
ACT(exp)-paced at ~1.1us/key-tile, so every other piece of PE work
(next-chunk Q/K projections, V-tile projections, previous-chunk output
projection) is drip-fed through a global work queue popped right after
each QK pair at ~2-matmul granularity, with per-chunk deadline forcing.
K=1 matmuls don't register as HAM activity, so a ~7us burst of K=128
junk matmuls un-throttles the PE clock at t=0. The activation-table map
is patched so Ln and Exp share one table set (the stock chooser burns
2x 1.28us ACT_TABLE_LOAD per softmax normalization). The final chunk's
projection alternates DVE adds with bias-matmul+ACT copies so the drain
isn't single-engine serialized. Output partials leave as bf16.
"""

import numpy as np
import ml_dtypes

B, S, D, H = 2, 2048, 1024, 16
HD = 64
N_CORES = 8
HPC = 4          # heads per core
P = 128          # partitions
SC = 512         # query-chunk size
NCH = S // SC    # 4 query chunks
KT = S // P      # 16 key tiles
KC = D // P      # 8 contraction chunks for the QKV projection

BF16 = ml_dtypes.bfloat16

_CACHED = None


def _patch_act_tables():
    """Force the ACT-table chooser to use natural_log_exp_and_others for
    both Exp and Ln (one table-set, zero mid-kernel reloads) by emptying
    the alternative homes in the table map the bass-side pass consults.
    Indices are preserved, so the act_func_set_id written into the BIR
    still names a real set containing the right functions."""
    import functools
    import concourse.hw_specs as hw
    import concourse.bacc as bacc

    if getattr(bacc.get_activation_tables, "_attn_patched", False):
        return
    orig = hw.get_activation_tables

    @functools.cache
    def patched(arch):
        t = dict(orig(arch))
        keep = "natural_log_exp_and_others"
        if keep in t:
            for name in ("exp_and_others", "exp_and_friends", "natural_log"):
                if name in t:
                    t[name] = set()
        return t

    patched._attn_patched = True
    hw.get_activation_tables = patched
    bacc.get_activation_tables = patched


def _emit(nc, tc, ctx, tiles_d):
    import concourse.bass as bass
    from concourse import mybir

    f32 = mybir.dt.float32
    bf16 = mybir.dt.bfloat16
    AF = mybir.ActivationFunctionType

    (hT_d, wq_d, wk_d, wv_d, wp_d, bqkv_d, bp_d, cmask_d, out_d) = tiles_d

    persist = ctx.enter_context(tc.tile_pool(name="persist", bufs=1))
    # PSUM budget (8 banks): qk double-buffer 2x[128,1024] = 4, filler
    # accumulators 2x[128,512] = 2, pv accumulator [128,1024] = 2.
    ps_qk = ctx.enter_context(tc.tile_pool(name="ps_qk", bufs=2, space="PSUM"))
    ps_fill = ctx.enter_context(tc.tile_pool(name="ps_fill", bufs=2, space="PSUM"))
    ps_pv = ctx.enter_context(tc.tile_pool(name="ps_pv", bufs=1, space="PSUM"))
    # ring sizes: the cross-emitted boundary iterations keep one extra
    # exp tile in flight, and the deferred norms hold rbb/ot_f a full
    # iteration longer - size the SBUF rings so their WAR waits are never
    # the binding constraint (SBUF has ~60KB of headroom here)
    expp = ctx.enter_context(tc.tile_pool(name="expp", bufs=12))
    otfp = ctx.enter_context(tc.tile_pool(name="otfp", bufs=3))
    otbp = ctx.enter_context(tc.tile_pool(name="otbp", bufs=6))
    rbp = ctx.enter_context(tc.tile_pool(name="rbp", bufs=4))
    outp = ctx.enter_context(tc.tile_pool(name="outp", bufs=6))

    # ---- HAM warm-up: K=128 junk matmuls. K=1 matmuls do not register
    # as PE activity, and the free-running 3.4us activity window must be
    # fully covered, so burn ~7us (16 x N=512 at the cold 1.2GHz rate).
    warm = persist.tile([P, SC], bf16, tag="warm", name="warm")
    nc.gpsimd.memset(warm[:], 0.25)
    for _ in range(15):
        junk = ps_fill.tile([P, SC], f32, tag="fill", name="junk")
        nc.tensor.matmul(junk[:], lhsT=warm[:, 0:P], rhs=warm[:],
                         start=True, stop=True, skip_group_check=True)

    # ---- persistent SBUF tensors / input DMAs ----
    # hT arrives as 4 sequence-chunk descriptors hcs[c] = hT[:, c*512:+512]
    # laid out [p, kc, s]; chunk-0 Q/K projections need only hcs[0].
    wq_sb = persist.tile([P, KC * 256], bf16, tag="wq", name="wq")
    wk_sb = persist.tile([P, KC * 256], bf16, tag="wk", name="wk")
    wv_sb = persist.tile([P, KC * 256], bf16, tag="wv", name="wv")
    bqkv_sb = persist.tile([P, 6], f32, tag="bqkv", name="bqkv")
    hcs = [persist.tile([P, KC * SC], bf16, tag=f"hcs{c}", name=f"hcs{c}")
           for c in range(NCH)]
    # chunk 0 additionally splits into four independent quarter-tiles so
    # the first Q/K-projection matmuls stream in behind each arriving
    # quarter instead of waiting for the whole chunk
    hc0q = [persist.tile([P, 2 * SC], bf16, tag=f"hc0q{q}", name=f"hc0q{q}")
            for q in range(4)]
    masks_all = persist.tile([P, 4 * 2 * SC], bf16, tag="masks", name="masks")
    wp_sb = persist.tile([P, 2 * D], bf16, tag="wp", name="wp")
    bp_sb = persist.tile([P, 2 * SC], bf16, tag="bp", name="bp")
    ones1 = persist.tile([1, P], bf16, tag="ones1", name="ones1")
    qt = [[persist.tile([P, SC], bf16, tag=f"qt{p}_{c}", name=f"qt{p}_{c}") for c in range(NCH)] for p in range(2)]
    kt = [[persist.tile([P, SC], bf16, tag=f"kt{p}_{c}", name=f"kt{p}_{c}") for c in range(NCH)] for p in range(2)]
    vt = [[persist.tile([P, 256], bf16, tag=f"vt{p}_{st}", name=f"vt{p}_{st}") for st in range(KT)] for p in range(2)]

    # All GpSimd memsets (V'' ones-blocks etc.) must precede the DMA gate
    # copies below in the GpSimd FIFO, or they'd queue behind the whole
    # input stream.
    nc.gpsimd.memset(ones1[:], 1.0)
    for p in range(2):
        for st in range(KT):
            vv = vt[p][st].rearrange("p (a b) -> p a b", a=2)
            nc.gpsimd.memset(vv[:, :, 64:128], 1.0)

    def hts(kc, c):
        if c == 0:
            return hc0q[kc // 2][:, (kc % 2) * SC:(kc % 2 + 1) * SC]
        return hcs[c][:, kc * SC:(kc + 1) * SC]

    # The DMA queues round-robin all in-flight descriptors, so issuing
    # everything up front makes the critical first chunk crawl at 1/N of
    # bandwidth. Chain the stream in consumption order: each later DMA's
    # destination gets a 1-element GpSimd copy FROM the previous stage's
    # tile, whose WAR dependency delays the descriptor until the previous
    # transfer finished.
    def _gate(dst_tile, src_tile):
        nc.gpsimd.tensor_copy(dst_tile[0:1, 0:1], src_tile[0:1, 0:1])

    nc.sync.dma_start(
        wq_sb[:].rearrange("p (a n) -> p a n", a=KC),
        wq_d.rearrange("(a p) n -> p a n", p=P),
    )
    nc.sync.dma_start(
        wk_sb[:].rearrange("p (a n) -> p a n", a=KC),
        wk_d.rearrange("(a p) n -> p a n", p=P),
    )
    nc.sync.dma_start(
        bqkv_sb[:].rearrange("p (a b) -> p a b", a=2),
        bqkv_d.rearrange("a p b -> p a b"),
    )
    # A single DMA descriptor only sustains ~100GB/s, and concurrent
    # descriptors fair-share the HBM pipe - so the critical hT chunk gets
    # WEIGHT by splitting into multiple descriptors, and later stages are
    # gated so they don't steal bandwidth from earlier ones.
    def _hc_dma(c, nsplit, eng):
        w = KC // nsplit  # kc chunks per descriptor
        for q in range(nsplit):
            eng.dma_start(
                hcs[c][:, q * w * SC:(q + 1) * w * SC].rearrange(
                    "p (a s) -> p a s", a=w),
                hT_d[q * w * P:(q + 1) * w * P, c * SC:(c + 1) * SC].rearrange(
                    "(a p) s -> p a s", p=P),
            )

    for q in range(4):
        nc.scalar.dma_start(
            hc0q[q][:].rearrange("p (a s) -> p a s", a=2),
            hT_d[q * 2 * P:(q + 1) * 2 * P, 0:SC].rearrange(
                "(a p) s -> p a s", p=P),
        )
    # stage 2 (after hc0): masks x2, wv, hc1 x2
    _gate(masks_all, hc0q[3])
    for dd in (0, 2):
        nc.sync.dma_start(
            masks_all[:, dd * 2 * SC:(dd + 2) * 2 * SC].rearrange(
                "p (d n) -> p d n", d=2),
            cmask_d[dd:dd + 2].rearrange("d p n -> p d n"),
        )
    _gate(wv_sb, hc0q[3])
    nc.sync.dma_start(
        wv_sb[:].rearrange("p (a n) -> p a n", a=KC),
        wv_d.rearrange("(a p) n -> p a n", p=P),
    )
    _gate(hcs[1], hc0q[3])
    _hc_dma(1, 2, nc.sync)
    # stage 3 (after hc1): hc2 x2, hc3 x2
    for c in (2, 3):
        _gate(hcs[c], hcs[1])
        _hc_dma(c, 2, nc.sync)
    # stage 4 (after hc3): wp, bp
    _gate(wp_sb, hcs[3])
    nc.sync.dma_start(
        wp_sb[:].rearrange("p (a n) -> p a n", a=2),
        wp_d.rearrange("(a p) n -> p a n", p=P),
    )
    _gate(bp_sb, hcs[3])
    nc.sync.dma_start(bp_sb[:], bp_d)

    masks = [masks_all[:, dd * 2 * SC:(dd + 1) * 2 * SC] for dd in range(4)]

    # ---- work queue: PE filler drip-fed into the ACT-paced k-loop ----
    # Items are (deadline_chunk, closure); closures sharing a PSUM
    # accumulator are queued consecutively (ps_fill holds at most 2 live
    # accumulators, and in-order popping guarantees that bound).
    work_q = []

    qkproj_done = {}  # (chunk, hpair) -> #completed units (of 2)

    def q_qkproj(c, hpairs=(0, 1), deadline=None):
        """Q^T/K^T for chunk c: per hpair 2 units x 4 sub-closures of 2
        matmuls."""
        if deadline is None:
            deadline = c
        for p in hpairs:
            for which in range(2):
                dst, w_sb, bcol = (
                    (qt, wq_sb, 0) if which == 0 else (kt, wk_sb, 1)
                )
                box = {}

                def sub(s, p=p, dst=dst, w_sb=w_sb, bcol=bcol, box=box, c=c):
                    if s == 0:
                        box["ps"] = ps_fill.tile([P, SC], f32, tag="fill", name="qkproj")
                    ps = box["ps"]
                    for kc in (2 * s, 2 * s + 1):
                        nc.tensor.matmul(
                            ps[:],
                            lhsT=w_sb[:, kc * 256 + 128 * p: kc * 256 + 128 * p + 128],
                            rhs=hts(kc, c),
                            start=(kc == 0), stop=(kc == KC - 1),
                            skip_group_check=True,
                        )
                    if s == 3:
                        nc.vector.tensor_scalar_add(
                            dst[p][c][:], ps[:],
                            bqkv_sb[:, 3 * p + bcol: 3 * p + bcol + 1])
                        qkproj_done[(c, p)] = qkproj_done.get((c, p), 0) + 1
                for s in range(4):
                    work_q.append((deadline, lambda s=s, sub=sub: sub(s)))

    def q_vproj(st, deadline):
        """V'' for key-tile st: 2 sub-closures of 4 matmuls (+copy)."""
        box = {}

        def sub(s, st=st, box=box):
            if s == 0:
                box["ps"] = ps_fill.tile([P, 256], f32, tag="fill", name="vproj")
            ps = box["ps"]
            for kc in range(4 * s, 4 * s + 4):
                nc.tensor.matmul(
                    ps[:],
                    lhsT=hts(kc, st // 4)[:, (st % 4) * P:(st % 4 + 1) * P],
                    rhs=wv_sb[:, kc * 256:(kc + 1) * 256],
                    start=(kc == 0), stop=(kc == KC - 1),
                    skip_group_check=True,
                )
            if s == 1:
                for p in range(2):
                    vv = vt[p][st].rearrange("p (a b) -> p a b", a=2)
                    nc.vector.tensor_copy(
                        vv[:, :, 0:64],
                        ps[:, 128 * p:128 * p + 128].rearrange("p (a b) -> p a b", a=2),
                    )
        for s in range(2):
            work_q.append((deadline, lambda s=s, sub=sub: sub(s)))

    def _proj_group(c, ots, st, dc, alt=False, drain=False, psw=None):
        """out[c*SC+st*128 : +128, dc*512 : +512] = ots @ W_proj + bias.
        alt=True: bias via K=1 matmul + ACT copy (drain load-balancing).
        drain=True: accumulate in the (now idle) qk pool - its slots were
        released by ACT exps long ago, while ps_fill's release waits sit
        behind the final norm's DVE ops in counter order. psw: caller-
        provided PSUM bank (drain groups share tiles pairwise so the ring
        is 4 banks deep and the matmul stream never waits on copies)."""
        if psw is None:
            if drain:
                ps = ps_qk.tile([P, 2 * SC], f32, tag="qksc", name="projd")
            else:
                ps = ps_fill.tile([P, SC], f32, tag="fill", name="proj")
            psw = ps[:, 0:SC]
        if alt:
            nc.tensor.matmul(
                psw, lhsT=ones1[:], rhs=bp_sb[0:1, dc * SC:(dc + 1) * SC],
                start=True, stop=False, skip_group_check=True,
            )
        for p in range(2):
            nc.tensor.matmul(
                psw,
                lhsT=ots[p][:, st * P:(st + 1) * P],
                rhs=wp_sb[:, p * D + dc * SC: p * D + (dc + 1) * SC],
                start=(p == 0 and not alt), stop=(p == 1),
                skip_group_check=True,
            )
        ob = outp.tile([P, SC], bf16, tag="ob", name="ob")
        if alt:
            nc.scalar.activation(ob[:], psw, AF.Copy, bias=0.0, scale=1.0)
        else:
            nc.vector.tensor_add(ob[:], psw, bp_sb[:, dc * SC:(dc + 1) * SC])
        # In the drain, descriptor-issue time (~0.6us each, serial per
        # engine) paces the tail - issue from GpSimd, which is idle there
        # (scalar/vector are busy with the copies themselves).
        eng = nc.gpsimd if drain else nc.sync
        eng.dma_start(
            out_d[c * SC + st * P: c * SC + (st + 1) * P, dc * SC:(dc + 1) * SC],
            ob[:],
        )

    def q_proj(c, ots, deadline, groups=None):
        for st in range(SC // P):
            for dc in range(2):
                if groups is not None and st * 2 + dc not in groups:
                    continue
                work_q.append((deadline, lambda c=c, ots=ots, st=st, dc=dc:
                               _proj_group(c, ots, st, dc)))

    def pop_work(n):
        for _ in range(n):
            if not work_q:
                return
            work_q.pop(0)[1]()

    def pop_deadline(chunk):
        while work_q and work_q[0][0] <= chunk:
            work_q.pop(0)[1]()

    # ---- stage A: chunk-0 projections (direct, stream behind the DMA) --
    def _qkproj_now(c, p, which):
        dst, w_sb, bcol = ((qt, wq_sb, 0) if which == 0 else (kt, wk_sb, 1))
        ps = ps_fill.tile([P, SC], f32, tag="fill", name="qkproj0")
        for kc in range(KC):
            nc.tensor.matmul(
                ps[:],
                lhsT=w_sb[:, kc * 256 + 128 * p: kc * 256 + 128 * p + 128],
                rhs=hts(kc, c),
                start=(kc == 0), stop=(kc == KC - 1),
                skip_group_check=True,
            )
        nc.vector.tensor_scalar_add(dst[p][c][:], ps[:], bqkv_sb[:, 3 * p + bcol: 3 * p + bcol + 1])

    def _vproj_now(st):
        ps = ps_fill.tile([P, 256], f32, tag="fill", name="vproj0")
        for kc in range(KC):
            nc.tensor.matmul(
                ps[:],
                lhsT=hts(kc, st // 4)[:, (st % 4) * P:(st % 4 + 1) * P],
                rhs=wv_sb[:, kc * 256:(kc + 1) * 256],
                start=(kc == 0), stop=(kc == KC - 1),
                skip_group_check=True,
            )
        for p in range(2):
            vv = vt[p][st].rearrange("p (a b) -> p a b", a=2)
            nc.vector.tensor_copy(
                vv[:, :, 0:64],
                ps[:, 128 * p:128 * p + 128].rearrange("p (a b) -> p a b", a=2),
            )

    # Minimal pre-loop: only what chunk-0 hpair-0's first exp needs.
    # Everything else (V tiles JIT in-loop, hpair-1 units via pops).
    _qkproj_now(0, 0, 0)
    _qkproj_now(0, 0, 1)
    q_qkproj(0, hpairs=(1,), deadline=0.5)

    # The softmax normalization is split: the Ln is emitted at the hpair
    # boundary, but the reciprocal-exp + rescale (normB) is deferred past
    # the next hpair's first exp so the ACT FIFO isn't stalled by the
    # Ln->Exp chain right when the next k-loop could already start.
    pending_norm = []

    def _norm_b(p, pvb, rbb):
        # 1/l = exp(-ln(l)) on ACT: both fns live in one table set (see
        # _patch_act_tables). DVE reciprocal costs ~3.3us/tile and the
        # fast custom-DVE approx returns garbage on this HW. The Ln is
        # IN the deferred closure: emitted at the boundary it would sit
        # in the ACT FIFO ahead of the next hpair's first exp, waiting
        # on the last PV - a ~1us ACT bubble at every hpair boundary.
        nc.scalar.activation(pvb[64:128, :], pvb[64:128, :], AF.Ln)
        nc.scalar.activation(rbb[64:128, :], pvb[64:128, :], AF.Exp, bias=0.0, scale=-1.0)
        ot_f = otfp.tile([P, SC], f32, tag="ot_f", name="ot_f")
        nc.vector.tensor_mul(ot_f[0:64, :], pvb[0:64, 0:SC], rbb[64:128, 0:SC])
        nc.vector.tensor_mul(ot_f[64:128, :], pvb[0:64, SC:2 * SC], rbb[64:128, SC:2 * SC])
        ot_b = otbp.tile([P, SC], bf16, tag="ot_b", name="ot_b")
        nc.vector.tensor_scalar_add(ot_b[:], ot_f[:], bqkv_sb[:, 3 * p + 2: 3 * p + 3])
        return ot_b

    def flush_norm():
        while pending_norm:
            pending_norm.pop(0)()

    # ---- stage B+C: attention + projection, per query chunk ----
    ots_by_chunk = [[None, None] for _ in range(NCH)]
    cross = None  # (ex, j0) of a boundary-crossing pre-emitted iteration
    for c in range(NCH):
        nt = 4 * (c + 1)  # causal: key tiles 0 .. 4c+3

        if c + 1 < NCH:
            # p0's projections must land before chunk c+1 starts; p1's
            # only before its second hpair - staggering the deadlines
            # halves the forced lump at each chunk boundary. V-tiles
            # 4(c+1)+2/+3 are emitted JIT inside chunk c+1 itself.
            q_qkproj(c + 1, hpairs=(0,), deadline=c + 1)
            for st in range(4 * (c + 1), 4 * (c + 1) + 2):
                q_vproj(st, c + 1)
            q_qkproj(c + 1, hpairs=(1,), deadline=c + 1.5)
        if c >= 1 and c < NCH - 1:
            q_proj(c - 1, ots_by_chunk[c - 1], c + 2)
        elif c == NCH - 1:
            # most of proj(NCH-2) feeds this chunk's pops (the queue runs
            # dry here otherwise and the idle PE re-throttles HAM); the
            # last two groups stay reserved for the drain's norm window
            q_proj(c - 1, ots_by_chunk[c - 1], c + 3, groups=range(6))

        def emit_qk_exp(p, t, cc=None):
            """QK pair + exp for (chunk cc, hpair p, key-tile t)."""
            if cc is None:
                cc = c
            j0 = P * (t - 4 * cc) if t >= 4 * cc else 0
            qk = ps_qk.tile([P, 2 * SC], f32, tag="qksc", name="qk")
            ktile = kt[p][t // 4]
            # scores^T[keys, queries] = K^T_tile.T @ Q^T_chunk
            nc.tensor.matmul(
                qk[:, j0:SC], lhsT=ktile[0:64, (t % 4) * P:(t % 4 + 1) * P],
                rhs=qt[p][cc][0:64, j0:SC], start=True, stop=True,
            )
            nc.tensor.matmul(
                qk[:, SC + j0:2 * SC], lhsT=ktile[64:128, (t % 4) * P:(t % 4 + 1) * P],
                rhs=qt[p][cc][64:128, j0:SC], start=True, stop=True,
            )
            ex = expp.tile([P, 2 * SC], bf16, tag="exp", name="exp")
            qk2v = qk.rearrange("p (a b) -> p a b", a=2)
            ex2v = ex.rearrange("p (a b) -> p a b", a=2)
            nc.scalar.activation(ex2v[:, :, j0:SC], qk2v[:, :, j0:SC], AF.Exp, bias=0.0, scale=0.125)
            return ex, j0

        for p in range(2):
            pvb = ps_pv.tile([P, 2 * SC], f32, tag="pv", name=f"pvb{p}")
            for ti, t in enumerate(range(nt)):
                if ti == 0 and cross is not None:
                    # QK+exp were cross-emitted during the previous
                    # hpair/chunk's last iteration (they run under its
                    # last exp, closing the ~1.2us ACT bubble of the
                    # exp->mask->PV->QK boundary chain)
                    ex, j0 = cross
                    cross = None
                    pop_work(2)
                else:
                    ex, j0 = None, None
                if ex is None:
                    ex, j0 = emit_qk_exp(p, t)
                    if c == 0 and p == 0:
                        # chunk 0: V'' for tile t JIT right before its PV
                        _vproj_now(t)
                    elif p == 0 and t in (1, 2):
                        # JIT V'' for this chunk's later diagonal tiles
                        # (consumed at t=4c+2 / 4c+3, safely ahead)
                        _vproj_now(4 * c + 1 + t)
                    else:
                        # higher pop rate early in each hpair replaces
                        # popping at the boundary itself (which would
                        # wedge filler ahead of the next hpair's QK).
                        # The filler budget is EXACTLY this: adding pops
                        # at ti==3, at the vproj iterations, or late in
                        # the chunk were each measured 3-8us WORSE - the
                        # exp stream's cushion is only ~2 iterations of
                        # deferred-norm backlog at each hpair start.
                        pop_work(2 if ti < 3 else 1)
                if ti == 1:
                    # deferred norm of the previous hpair: two exps of
                    # this hpair are already in the ACT FIFO ahead of the
                    # Ln, so by the time ACT reaches it the last PV (its
                    # input) is long done - no FIFO-head stall
                    flush_norm()
                if p == 0 and ti == nt - 1:
                    # cross-emit the next hpair's first QK+exp; any
                    # leftover producers of qt/kt[1] must be forced out
                    # first or the QK would deadlock behind them
                    pop_deadline(c + 0.5)
                    cross = emit_qk_exp(1, 0)
                elif p == 1 and ti == nt - 1 and c + 1 < NCH:
                    # same across the chunk boundary. If the next chunk's
                    # p0 Q/K units already popped, cross-emit BEFORE the
                    # deadline force so the leftover filler lump doesn't
                    # delay the QK; otherwise the producers must precede
                    # it in the PE FIFO (deadlock otherwise).
                    if qkproj_done.get((c + 1, 0), 0) == 2:
                        cross = emit_qk_exp(0, 0, cc=c + 1)
                        pop_deadline(c + 1)
                    else:
                        pop_deadline(c + 1)
                        cross = emit_qk_exp(0, 0, cc=c + 1)
                if t >= 4 * c:  # diagonal tile: causal mask
                    ex2v = ex.rearrange("p (a b) -> p a b", a=2)
                    exm = expp.tile([P, 2 * SC], bf16, tag="exp", name="exm")
                    nc.vector.tensor_mul(
                        exm.rearrange("p (a b) -> p a b", a=2)[:, :, j0:SC],
                        ex2v[:, :, j0:SC],
                        masks[t - 4 * c].rearrange("p (a b) -> p a b", a=2)[:, :, j0:SC],
                    )
                    ex = exm
                last = (ti == nt - 1)
                nc.tensor.matmul(pvb[:, j0:SC], lhsT=vt[p][t][:, 0:128], rhs=ex[:, j0:SC],
                                 start=(ti == 0), stop=last, skip_group_check=True)
                nc.tensor.matmul(pvb[:, SC + j0:2 * SC], lhsT=vt[p][t][:, 128:256], rhs=ex[:, SC + j0:2 * SC],
                                 start=(ti == 0), stop=last, skip_group_check=True)

            # The whole normalization is deferred past the next hpair's
            # first exp (see _norm_b).
            rbb = rbp.tile([P, 2 * SC], f32, tag="rb", name="rbb")

            def _fin(p=p, pvb=pvb, rbb=rbb, c=c):
                ots_by_chunk[c][p] = _norm_b(p, pvb, rbb)
            pending_norm.append(_fin)

        # anything chunk c+1 consumes must be emitted before its k-loop
        pop_deadline(c + 1)

    # ---- drain: proj(NCH-2) emitted FIRST so its matmuls overlap the
    # last hpair's normalization chain (emitting it after flush_norm
    # would order its pool-reuse semaphore waits behind the normB DVE
    # ops - counter-based ordering), then normB, then proj(NCH-1) ----
    drain_groups = [(NCH - 2, g) for g in range(6, 8)] + \
                   [(NCH - 1, g) for g in range(8)]
    tile_box = None
    emitted = 0

    def _drain_group(cg, g):
        nonlocal tile_box, emitted
        if emitted % 2 == 0:
            tile_box = ps_qk.tile([P, 2 * SC], f32, tag="qksc", name="projd")
            psw = tile_box[:, 0:SC]
        else:
            psw = tile_box[:, SC:2 * SC]
        emitted += 1
        _proj_group(cg, ots_by_chunk[cg], g // 2, g % 2,
                    alt=(g % 2 == 0), drain=True, psw=psw)

    for cg, g in drain_groups[:2]:
        _drain_group(cg, g)
    flush_norm()
    pop_deadline(NCH + 1)
    for cg, g in drain_groups[2:]:
        _drain_group(cg, g)


def build():
    from contextlib import ExitStack
    import concourse.tile as tile
    from concourse import bacc, mybir

    _patch_act_tables()

    f32 = mybir.dt.float32
    bf16 = mybir.dt.bfloat16

    nc = bacc.Bacc("TRN2", target_bir_lowering=False, debug=False, num_devices=N_CORES)
    hT_d = nc.dram_tensor("ht", [D, S], bf16, kind="ExternalInput").ap()
    wq_d = nc.dram_tensor("wq", [D, 256], bf16, kind="ExternalInput").ap()
    wk_d = nc.dram_tensor("wk", [D, 256], bf16, kind="ExternalInput").ap()
    wv_d = nc.dram_tensor("wv", [D, 256], bf16, kind="ExternalInput").ap()
    wp_d = nc.dram_tensor("wp", [256, D], bf16, kind="ExternalInput").ap()
    bqkv_d = nc.dram_tensor("bqkv", [2, P, 3], f32, kind="ExternalInput").ap()
    bp_d = nc.dram_tensor("bp", [P, 2 * SC], bf16, kind="ExternalInput").ap()
    cmask_d = nc.dram_tensor("cmask", [4, P, 2 * SC], bf16, kind="ExternalInput").ap()
    out_d = nc.dram_tensor("out", [S, D], bf16, kind="ExternalOutput").ap()

    with tile.TileContext(nc) as tc:
        with ExitStack() as ctx:
            _emit(nc, tc, ctx, (hT_d, wq_d, wk_d, wv_d, wp_d, bqkv_d, bp_d, cmask_d, out_d))
    nc.compile()
    return nc


def make_in_maps(hidden_states, W_attn, b_attn, W_proj, b_proj):
    hidden_states = np.asarray(hidden_states, dtype=np.float32)
    W_attn = np.asarray(W_attn, dtype=np.float32)
    b_attn = np.asarray(b_attn, dtype=np.float32)
    W_proj = np.asarray(W_proj, dtype=np.float32)
    b_proj = np.asarray(b_proj, dtype=np.float32)

    pp, jj = np.meshgrid(np.arange(P), np.arange(SC), indexing="ij")
    cmask1 = np.stack([(pp + P * dd <= jj) for dd in range(4)]).astype(np.float32)
    cmask = np.concatenate([cmask1, cmask1], axis=-1).astype(BF16)

    in_maps = []
    for core in range(N_CORES):
        b, g = divmod(core, 4)
        h0 = g * 256  # first local column (4 heads x 64)
        hT = np.ascontiguousarray(hidden_states[b].T).astype(BF16)
        wq = W_attn[:, h0:h0 + 256].astype(BF16)
        wk = W_attn[:, D + h0:D + h0 + 256].astype(BF16)
        wv = W_attn[:, 2 * D + h0:2 * D + h0 + 256].astype(BF16)
        wp = W_proj[h0:h0 + 256, :].astype(BF16)
        bqkv = np.empty((2, P, 3), np.float32)
        for p in range(2):
            lo = h0 + 128 * p
            bqkv[p, :, 0] = b_attn[lo:lo + 128]
            bqkv[p, :, 1] = b_attn[D + lo:D + lo + 128]
            bqkv[p, :, 2] = b_attn[2 * D + lo:2 * D + lo + 128]
        bp1 = b_proj if g == 0 else np.zeros_like(b_proj)
        bp_rep = np.ascontiguousarray(
            np.broadcast_to(bp1.astype(BF16)[None, :], (P, D))
        )
        in_maps.append({
            "ht": hT, "wq": wq, "wk": wk, "wv": wv, "wp": wp,
            "bqkv": bqkv, "bp": bp_rep,
            "cmask": cmask,
        })
    return in_maps


def _run(in_maps, trace=False):
    global _CACHED
    from concourse.bass_utils import run_bass_kernel_spmd

    if _CACHED is None:
        _CACHED = build()
    res = run_bass_kernel_spmd(
        _CACHED, in_maps, core_ids=list(range(N_CORES)), trace=trace
    )
    out = np.zeros((B, S, D), np.float32)
    for core in range(N_CORES):
        out[core // 4] += np.asarray(res.results[core]["out"], dtype=np.float32)
    return out, res


def kernel(hidden_states, W_attn, b_attn, W_proj, b_proj):
    in_maps = make_in_maps(hidden_states, W_attn, b_attn, W_proj, b_proj)
    out, _ = _run(in_maps)
    return out


def run_profiled(hidden_states, W_attn, b_attn, W_proj, b_proj):
    """Like kernel(), but captures an NTFF profile; returns (out, exec_time_ns, res)."""
    in_maps = make_in_maps(hidden_states, W_attn, b_attn, W_proj, b_proj)
    out, res = _run(in_maps, trace=True)
    return out, res.exec_time_ns, res
